# revision 12
# baseline (speedup 1.0000x reference)
"""Contrastive loss kernel for Trainium2 (8 NeuronCores, Bass/Tile).

Math: with L2-normalized embeddings, dist = 1 - sim and MARGIN = 2.0, the
negative branch relu(2 - dist) = 1 + sim is never clipped (|sim| <= 1), so

    pair_loss = (1+sim)^2 - 4*sim*[same]

Summing the strict upper triangle of the symmetric pair matrix:

    total = (B^2 + 2*||s||^2 + ||C||_F^2 - 4*sum_k ||g_k||^2)/2

where C = E^T E (DxD), g_k = sum_{key_i=k} e_i (128 key groups), s = sum_i e_i.
Uses sum_ij sim^2 = tr((E^T E)^2) = ||C||_F^2, turning O(B^2 D) into O(B D^2).

Distribution: an 8-core AllReduce on this fabric costs ~57us -- far more than
the whole computation -- so every core redundantly computes the full reduction
(exec time is the max over cores, so redundancy is free) and core 0's scalar
is returned. The loss is dominated by the B^2 constant; the data-dependent
term contributes ~0.4% of its value, so fp8e4 inputs (rel err ~3%) perturb the
loss by ~1e-5 relative -- far inside the 2e-2 gate. fp8 quarters the DMA bytes
vs f32 and unlocks the PE's DoubleRow mode (256 contraction rows per
instruction, 1 cycle per output column).

Host prep packs F = [E | onehot(keys)] as fp8 in the exact SBUF layout
[128 partitions, 64 row-tiles, 384], so each chunk DMA is 128 descriptors of
3 KB contiguous on both sides (vs 8192x1KB for the f32 rearrange path). Per
row-tile-pair the two accumulating matmuls (F[:,:,0:128])^T F and
(F[:,:,128:256])^T F[:,:,128:384] yield [C00|C01|G0^T] and [C11|G1^T].

Schedule notes: chunk DMAs alternate the SP/Activation HWDGE queues so
descriptor prep of chunk c+1 overlaps the transfer of chunk c (a single queue
serializes prep with transfer, pacing the stream at ~1.8us/chunk instead of
~1.1). A zero warm-up matmul burst during the otherwise-idle DMA head ramps
the PE out of its 1.2 GHz cold p-state before the real stream arrives. Finals
accumulators live in one [128,16] tile (column slices) to minimize tile
semaphores -- the TileContext end-barrier cost scales with them.
"""

import sys

for _p in ("/opt/trn_rl_repo",):
    if _p not in sys.path:
        sys.path.insert(0, _p)

import ml_dtypes
import numpy as np

import concourse.bass as bass
import concourse.bacc as bacc
import concourse.mybir as mybir
import concourse.tile as tile
from concourse.bass_utils import run_bass_kernel_spmd

B, D = 8192, 256
N_CORES = 8
NKEYS = 128
NUM_PAIRS = B * (B - 1) // 2
NT = B // 128            # 64 row-tiles of 128 rows
NCHUNK = 8               # DMA granularity
TPC = NT // NCHUNK       # row-tiles per chunk
FW = D + NKEYS           # 384: [E | onehot] concat width. The dual-fp8
                         # LdWeights ISA rule rejects k-tile strides that are
                         # not a multiple of 128 (385 and 388 both fail), so
                         # no ones-column: s comes from row-reduces instead.
P1W = FW - 128           # 256: width of the second matmul chain

F32 = mybir.dt.float32
FP8 = mybir.dt.float8e4
NP_FP8 = ml_dtypes.float8_e4m3

_cache = {}


def _build():
    nc = bacc.Bacc(
        "TRN2",
        target_bir_lowering=False,
        debug=False,
        num_devices=N_CORES,
    )

    # uint8 at the host/PJRT boundary (fp8 transfers are not supported there);
    # bitcast to fp8e4 for the device-side view.
    fmat_u8 = nc.dram_tensor(
        "fmat", [128, NT, FW], mybir.dt.uint8, kind="ExternalInput"
    ).ap()
    fmat = fmat_u8.bitcast(FP8)
    loss_out = nc.dram_tensor("loss", [1, 1], F32, kind="ExternalOutput").ap()

    DR = mybir.MatmulPerfMode.DoubleRow
    ADD = mybir.AluOpType.add
    AX = mybir.AxisListType.X

    with tile.TileContext(nc) as tc:
        with (
            tc.tile_pool(name="work", bufs=1) as pool,
            tc.tile_pool(name="psum", bufs=1, space="PSUM") as psum,
        ):
            # PE p-state warm-up on zeros during the DMA head (results unread).
            warm = pool.tile([128, 2, FW], FP8)
            nc.vector.memset(warm[:], 0.0)
            pw = psum.tile([128, FW], F32, name="pw")
            for _ in range(6):
                nc.tensor.matmul(
                    pw[:], lhsT=warm[:, :, 0:128], rhs=warm[:, :, :],
                    start=True, stop=True, perf_mode=DR,
                )

            # p0 = [C00 | C01 | G^T rows 0:128 | s rows 0:128]
            # p1 = [C11 | G^T rows 128:256 | s rows 128:256]
            p0 = psum.tile([128, FW], F32, name="p0")
            p1 = psum.tile([128, P1W], F32, name="p1")

            for c in range(NCHUNK):
                fch = pool.tile([128, TPC, FW], FP8, tag="fch", bufs=4)
                dma_eng = nc.sync if c % 2 == 0 else nc.scalar
                dma_eng.dma_start(fch[:], fmat[:, c * TPC : (c + 1) * TPC, :])
                for j in range(0, TPC, 2):
                    gi = c * TPC + j
                    first, last = gi == 0, gi == NT - 2
                    nc.tensor.matmul(
                        p0[:], lhsT=fch[:, j : j + 2, 0:128],
                        rhs=fch[:, j : j + 2, :],
                        start=first, stop=last, perf_mode=DR,
                    )
                    nc.tensor.matmul(
                        p1[:], lhsT=fch[:, j : j + 2, 128:256],
                        rhs=fch[:, j : j + 2, 128:FW],
                        start=first, stop=last, perf_mode=DR,
                    )

            ones_sb = pool.tile([128, 1], F32)
            nc.vector.memset(ones_sb[:], 1.0)

            # ---- finals (all DVE; one accumulator tile to minimize tile
            # semaphores). acc columns:
            #  0 aC00  1 aC01  2 aC11  3 aG0  4 aG1  5 s0  6 s1
            #  7..17 squares and combination temps
            r0 = pool.tile([128, FW], F32)
            nc.vector.tensor_copy(r0[:], p0[:])
            r1 = pool.tile([128, P1W], F32)
            nc.vector.tensor_copy(r1[:], p1[:])

            acc = pool.tile([128, 24], F32)
            sq = pool.tile([128, 128], F32)
            for k, src in enumerate((
                r0[:, 0:128],      # C00
                r0[:, 128:256],    # C01
                r1[:, 0:128],      # C11
                r0[:, 256:384],    # G0^T
                r1[:, 128:256],    # G1^T
            )):
                nc.vector.tensor_mul(sq[:], src, src)
                nc.vector.tensor_reduce(acc[:, k : k + 1], sq[:], axis=AX, op=ADD)
            nc.vector.tensor_reduce(acc[:, 5:6], r0[:, 256:384], axis=AX, op=ADD)
            nc.vector.tensor_reduce(acc[:, 6:7], r1[:, 128:256], axis=AX, op=ADD)
            nc.vector.tensor_mul(acc[:, 7:8], acc[:, 5:6], acc[:, 5:6])
            nc.vector.tensor_mul(acc[:, 8:9], acc[:, 6:7], acc[:, 6:7])

            # comb = aC00 + 2*aC01 + aC11 - 4*(aG0+aG1) + 2*(ssq0+ssq1)
            nc.vector.tensor_scalar_mul(acc[:, 9:10], acc[:, 1:2], 2.0)
            nc.vector.tensor_add(acc[:, 10:11], acc[:, 0:1], acc[:, 9:10])
            nc.vector.tensor_add(acc[:, 11:12], acc[:, 10:11], acc[:, 2:3])
            nc.vector.tensor_add(acc[:, 12:13], acc[:, 3:4], acc[:, 4:5])
            nc.vector.tensor_scalar_mul(acc[:, 13:14], acc[:, 12:13], -4.0)
            nc.vector.tensor_add(acc[:, 14:15], acc[:, 7:8], acc[:, 8:9])
            nc.vector.tensor_scalar_mul(acc[:, 15:16], acc[:, 14:15], 2.0)
            nc.vector.tensor_add(acc[:, 16:17], acc[:, 11:12], acc[:, 13:14])
            nc.vector.tensor_add(acc[:, 17:18], acc[:, 16:17], acc[:, 15:16])

            # t1 = sum_p comb[p] via ones matmul, then affine to the loss.
            t1 = psum.tile([1, 1], F32, name="t1")
            nc.tensor.matmul(
                t1[:], lhsT=acc[:, 17:18], rhs=ones_sb[:], start=True, stop=True
            )
            fin = pool.tile([1, 2], F32)
            nc.vector.tensor_scalar_mul(fin[:, 0:1], t1[:], 1.0 / (2.0 * NUM_PAIRS))
            nc.vector.tensor_scalar_add(
                fin[:, 1:2], fin[:, 0:1], float(B) * B / (2.0 * NUM_PAIRS)
            )
            nc.sync.dma_start(loss_out[:], fin[:, 1:2])

    nc.compile()
    return nc


def _get_nc():
    if "nc" not in _cache:
        _cache["nc"] = _build()
    return _cache["nc"]


def _pack(embeddings: np.ndarray, order_keys: np.ndarray) -> np.ndarray:
    """[E | onehot(keys)] as fp8 bytes in SBUF layout [128, NT, FW]:
    fmat[p, t, :] = row t*128 + p."""
    emb8 = np.ascontiguousarray(embeddings, dtype=np.float32).astype(NP_FP8)
    onehot = np.zeros((B, NKEYS), dtype=NP_FP8)
    onehot[np.arange(B), order_keys.astype(np.int64)] = 1.0
    f = np.concatenate([emb8.view(np.uint8), onehot.view(np.uint8)], axis=1)
    return np.ascontiguousarray(f.reshape(NT, 128, FW).transpose(1, 0, 2))


def _in_maps(embeddings: np.ndarray, order_keys: np.ndarray):
    f = _pack(embeddings, order_keys)
    return [{"fmat": f} for _ in range(N_CORES)]


def kernel(embeddings: np.ndarray, order_keys: np.ndarray) -> np.ndarray:
    nc = _get_nc()
    res = run_bass_kernel_spmd(nc, _in_maps(embeddings, order_keys), list(range(N_CORES)))
    return np.asarray(res.results[0]["loss"], dtype=np.float32).reshape(())
